# revision 1
# baseline (speedup 1.0000x reference)
"""Trainium2 Bass kernel for nn_CrossAttention (dense_transformer).

Reference computation (N=4, C=512, H=W=32, NH=8 heads):
  k/q/v = 3x3 conv(x) for both branches (l, g); heads split the flattened
  spatial dim into 8 chunks of 128; attention uses channels (512) as the
  sequence axis and the 128 spatial positions as the head dim; then a 1x1
  conv and a scaled residual add.

Sharding: spatial. Head h covers image rows 4h..4h+3, so sharding H into
8 blocks of 4 rows makes every head fully core-local: core h computes the
convs for its 4 output rows (6 input rows with halo), its head's full
attention, the 1x1 conv, and the residual — zero cross-core communication.

Per-core kernel layout choices:
  * 3x3 conv as 36 accumulated matmuls (9 taps x 4 cin chunks), bf16 in,
    fp32 psum.  Q/K convs produce a transposed [spatial, channel] layout
    directly (lhsT = shifted x tile, rhs = weight [cin, cout]) so the
    attention QK^T matmul needs no transposes. V convs produce the normal
    [channel, spatial] layout (rhs operand of the AV matmul).
  * Softmax without max-subtraction (logits are within +-20 for this
    problem, exp is safe in fp32) and without an explicit row-sum pass:
    V is augmented with a ones column so the AV matmul's last output
    column is the softmax denominator ("ones trick").
  * resweight and the 1/sqrt(128) attention scale are folded into the 1x1
    conv weights / Q conv weights on the host (both enter linearly).
"""

import numpy as np
import ml_dtypes

N, C, H, W, NH = 4, 512, 32, 32, 8
P = 128
ROWS = H // NH          # 4 output rows per core
S = ROWS * W            # 128 spatial positions per core (= head dim)
CK = C // P             # 4 channel chunks
RP, WP_ = ROWS + 2, W + 2  # padded slice dims: 6 x 34
N_CORES = 8

_BUILT = {}


def _build_bass():
    import concourse.tile as tile
    import concourse.mybir as mybir
    from concourse import bacc

    f32 = mybir.dt.float32
    bf16 = mybir.dt.bfloat16
    AF = mybir.ActivationFunctionType

    nc = bacc.Bacc("TRN2", target_bir_lowering=False)

    xl_d = nc.dram_tensor("xl", [N, C, RP, WP_], f32, kind="ExternalInput")
    xg_d = nc.dram_tensor("xg", [N, C, RP, WP_], f32, kind="ExternalInput")
    xlb_d = nc.dram_tensor("xlb", [N, C, RP, WP_], bf16, kind="ExternalInput")
    xgb_d = nc.dram_tensor("xgb", [N, C, RP, WP_], bf16, kind="ExternalInput")
    # conv order: 0=Wk1(k_l) 1=Wv1(v_l) 2=Wq2*s(q_g) 3=Wq1*s(q_l) 4=Wk2(k_g) 5=Wv2(v_g)
    w3_d = nc.dram_tensor("w3", [6, 9, C, C], bf16, kind="ExternalInput")
    # wp[0]=Wp1*rw (local branch), wp[1]=Wp2*rw (global branch); layout [cin, cout]
    wp_d = nc.dram_tensor("wp", [2, C, C], bf16, kind="ExternalInput")
    outl_d = nc.dram_tensor("out_l", [N, C, S], f32, kind="ExternalOutput")
    outg_d = nc.dram_tensor("out_g", [N, C, S], f32, kind="ExternalOutput")

    with tile.TileContext(nc) as tc:
        with (
            tc.tile_pool(name="singles", bufs=1) as singles,
            tc.tile_pool(name="wpool", bufs=14) as wpool,
            tc.tile_pool(name="attp", bufs=2) as attp,
            tc.tile_pool(name="small", bufs=8) as small,
            tc.tile_pool(name="ps", bufs=5, space="PSUM") as ps_pool,
            tc.tile_pool(name="psy", bufs=2, space="PSUM") as psy_pool,
        ):
            # ---- load x: bf16 (host-cast) feeds the convs immediately;
            # fp32 arrives later, only needed for the residual adds.
            # layout [P, N, CK, rc]: (n, ck) merge on the DRAM side (3-dim DMA cap)
            xf = {}
            xb = {}
            for nm, tb, tf in (("l", xlb_d, xl_d), ("g", xgb_d, xg_d)):
                xb[nm] = singles.tile([P, N, CK, RP * WP_], bf16, tag=f"xb_{nm}", name=f"xb_{nm}")
                nc.sync.dma_start(
                    xb[nm], tb.rearrange("n (ck p) r c -> p n ck (r c)", p=P)
                )
                xf[nm] = singles.tile([P, N, CK, RP * WP_], f32, tag=f"xf_{nm}", name=f"xf_{nm}")
                nc.sync.dma_start(
                    xf[nm], tf.rearrange("n (ck p) r c -> p n ck (r c)", p=P)
                )

            def conv3x3(conv_idx, xsrc, consume):
                """3x3 valid conv, normal [cout_chunk, (n, s)] psum layout.

                consume(co, psum) copies each cout-chunk psum out."""
                w_taps = []
                for tap in range(9):
                    wt = wpool.tile([P, CK, C], bf16, tag="w3", name="w3")
                    # scalar-engine HWDGE queue: keeps the SP queue free for
                    # x/out traffic; Bacc splits multi-sem waits for us
                    nc.scalar.dma_start(
                        wt, w3_d[conv_idx, tap].rearrange("(ck p) co -> p ck co", p=P)
                    )
                    w_taps.append(wt)
                for co in range(CK):
                    psum = ps_pool.tile([P, C], f32, tag="ps", name="ps")
                    pview = psum.rearrange("p (n r c) -> p n r c", n=N, r=ROWS)
                    first = True
                    for tap in range(9):
                        ky, kx = tap // 3, tap % 3
                        for ck in range(CK):
                            xs = xsrc[:, :, ck].rearrange("p n (r c) -> p n r c", c=WP_)
                            rhs = xs[:, :, ky:ky + ROWS, kx:kx + W]   # [P, 4, 4, 32]
                            nc.tensor.matmul(
                                pview,
                                lhsT=w_taps[tap][:, ck, co * P:(co + 1) * P],
                                rhs=rhs,
                                start=first,
                                stop=(tap == 8 and ck == CK - 1),
                            )
                            first = False
                    consume(co, psum)

            def conv_N(conv_idx, xsrc, vdst):
                """conv into v_aug [ck, n, s] (normal layout + ones column)."""
                def consume(co, psum):
                    nc.vector.tensor_copy(
                        out=vdst[:, co, :, 0:S],
                        in_=psum.rearrange("p (n s) -> p n s", n=N),
                    )
                conv3x3(conv_idx, xsrc, consume)

            def conv_T(conv_idx, xsrc, dst):
                """conv into transposed [s, (n, c)] layout via PE transpose."""
                def consume(co, psum):
                    qn = attp.tile([P, N, S], bf16, tag="qn", name="qn", bufs=4)
                    nc.vector.tensor_copy(
                        out=qn, in_=psum.rearrange("p (n s) -> p n s", n=N)
                    )
                    for n in range(N):
                        pt = psy_pool.tile([P, P], bf16, tag="pt", name="pt",
                                           bufs=1)
                        nc.tensor.transpose(pt, qn[:, n, :], identity)
                        nc.vector.tensor_copy(
                            out=dst[:, n, co * P:(co + 1) * P], in_=pt
                        )
                conv3x3(conv_idx, xsrc, consume)

            def attend_qk(q_sb, k_sb, tag):
                """QK^T + exp, per batch; returns pexp tiles [ck, cq]."""
                pexps = []
                for n in range(N):
                    pexp = attp.tile([P, CK, C], bf16, tag=f"pexp_{tag}",
                                     name=f"pexp_{tag}", bufs=4)
                    for ck in range(CK):
                        psum = ps_pool.tile([P, C], f32, tag="ps", name="ps")
                        nc.tensor.matmul(
                            psum,
                            lhsT=k_sb[:, n, ck * P:(ck + 1) * P],
                            rhs=q_sb[:, n, :],
                            start=True,
                            stop=True,
                        )
                        nc.scalar.activation(
                            out=pexp[:, ck, :], in_=psum, func=AF.Exp
                        )
                    pexps.append(pexp)
                return pexps

            def attend_av(pexps, vaug, wp_idx, xres, out_d, out_tag):
                y_sb = singles.tile([P, CK, N, S], bf16, tag=f"y_{out_tag}", name=f"y_{out_tag}")
                # cq outer: y[:, cq] finishes early so the 1x1 conv's first
                # accumulations overlap the remaining AV matmuls
                for cq in range(CK):
                    for n in range(N):
                        pexp = pexps[n]
                        psy = psy_pool.tile([P, S + 1], f32, tag="psy", name="psy")
                        for ck in range(CK):
                            nc.tensor.matmul(
                                psy,
                                lhsT=pexp[:, ck, cq * P:(cq + 1) * P],
                                rhs=vaug[:, ck, n, :],
                                start=(ck == 0),
                                stop=(ck == CK - 1),
                            )
                        rec = small.tile([P, 1], f32, tag="rec", name="rec")
                        nc.vector.reciprocal(rec, psy[:, S:S + 1])
                        nc.scalar.activation(
                            out=y_sb[:, cq, n, :],
                            in_=psy[:, 0:S],
                            func=AF.Copy,
                            scale=rec,
                        )
                # 1x1 conv (weights pre-scaled by resweight) + residual add
                wp_sb = singles.tile([P, CK, C], bf16, tag=f"wp_{out_tag}", name=f"wp_{out_tag}")
                nc.sync.dma_start(
                    wp_sb, wp_d[wp_idx].rearrange("(ck p) co -> p ck co", p=P)
                )
                out_sb = singles.tile([P, N, CK, S], f32, tag=f"out_{out_tag}", name=f"out_{out_tag}")
                for co in range(CK):
                    psum = ps_pool.tile([P, C], f32, tag="ps", name="ps")
                    for ci in range(CK):
                        nc.tensor.matmul(
                            psum,
                            lhsT=wp_sb[:, ci, co * P:(co + 1) * P],
                            rhs=y_sb[:, ci].rearrange("p n s -> p (n s)"),
                            start=(ci == 0),
                            stop=(ci == CK - 1),
                        )
                    xr = xres[:, :, co].rearrange("p n (r c) -> p n r c", c=WP_)
                    nc.vector.tensor_add(
                        out=out_sb[:, :, co].rearrange("p n (r c) -> p n r c", c=W),
                        in0=psum.rearrange("p (n r c) -> p n r c", n=N, r=ROWS),
                        in1=xr[:, :, 1:1 + ROWS, 1:1 + W],
                    )
                nc.sync.dma_start(
                    out_d.rearrange("n (co p) s -> p n co s", p=P), out_sb
                )

            # identity for PE-mode transpose
            identity = singles.tile([P, P], bf16, tag="identity", name="identity")
            from concourse.masks import make_identity
            make_identity(nc, identity)

            # persistent attention operands
            k1 = singles.tile([P, N, C], bf16, tag="k1", name="k1")   # k_l  [s, n, c]
            q2 = singles.tile([P, N, C], bf16, tag="q2", name="q2")   # q_g
            q1 = singles.tile([P, N, C], bf16, tag="q1", name="q1")   # q_l
            k2 = singles.tile([P, N, C], bf16, tag="k2", name="k2")   # k_g
            vl = singles.tile([P, CK, N, S + 1], bf16, tag="vl", name="vl")
            vg = singles.tile([P, CK, N, S + 1], bf16, tag="vg", name="vg")
            nc.vector.memset(vl[:, :, :, S:S + 1], 1.0)
            nc.vector.memset(vg[:, :, :, S:S + 1], 1.0)

            # branch A: global queries attend over local k/v -> out_g
            # branch B: local queries attend over global k/v -> out_l
            # order: QK of each branch as soon as its q/k convs finish, so
            # only the AV+proj of branch B sits in the kernel tail.
            conv_T(0, xb["l"], k1)
            conv_T(2, xb["g"], q2)
            pexps_a = attend_qk(q2, k1, "a")
            conv_N(1, xb["l"], vl)
            attend_av(pexps_a, vl, 1, xf["g"], outg_d, "g")
            conv_T(3, xb["l"], q1)
            conv_T(4, xb["g"], k2)
            pexps_b = attend_qk(q1, k2, "b")
            conv_N(5, xb["g"], vg)
            attend_av(pexps_b, vg, 0, xf["l"], outl_d, "l")

    nc.finalize()
    return nc


def _prep_host_inputs(x_l, x_g, Wk1, Wq1, Wv1, Wk2, Wq2, Wv2, Wp1, Wp2, resweight):
    """Build the 8 per-core input maps (numpy) from full inputs."""
    bf = ml_dtypes.bfloat16
    scale = 1.0 / np.sqrt(np.float32(S))
    rw = np.float32(np.asarray(resweight))

    def t3(w, s=1.0):
        # [cout, cin, 3, 3] -> [9, cin, cout] bf16
        w = np.asarray(w, np.float32) * s
        return np.ascontiguousarray(
            w.transpose(2, 3, 1, 0).reshape(9, C, C)
        ).astype(bf)

    w3 = np.stack([
        t3(Wk1), t3(Wv1), t3(Wq2, scale), t3(Wq1, scale), t3(Wk2), t3(Wv2),
    ])  # [6, 9, C, C]
    wp = np.stack([
        np.ascontiguousarray(np.asarray(Wp1, np.float32)[:, :, 0, 0].T * rw),
        np.ascontiguousarray(np.asarray(Wp2, np.float32)[:, :, 0, 0].T * rw),
    ]).astype(bf)  # [2, C(in), C(out)]

    xl_p = np.pad(np.asarray(x_l, np.float32), ((0, 0), (0, 0), (1, 1), (1, 1)))
    xg_p = np.pad(np.asarray(x_g, np.float32), ((0, 0), (0, 0), (1, 1), (1, 1)))

    in_maps = []
    for core in range(N_CORES):
        r0 = core * ROWS
        xl_s = np.ascontiguousarray(xl_p[:, :, r0:r0 + RP, :])
        xg_s = np.ascontiguousarray(xg_p[:, :, r0:r0 + RP, :])
        in_maps.append({
            "xl": xl_s,
            "xg": xg_s,
            "xlb": xl_s.astype(bf),
            "xgb": xg_s.astype(bf),
            "w3": w3,
            "wp": wp,
        })
    return in_maps


def kernel(x_l, x_g, Wk1, Wq1, Wv1, Wk2, Wq2, Wv2, Wp1, Wp2, resweight,
           _trace=False):
    from concourse.bass_utils import run_bass_kernel_spmd

    if "nc" not in _BUILT:
        _BUILT["nc"] = _build_bass()
    nc = _BUILT["nc"]

    in_maps = _prep_host_inputs(
        x_l, x_g, Wk1, Wq1, Wv1, Wk2, Wq2, Wv2, Wp1, Wp2, resweight
    )
    res = run_bass_kernel_spmd(
        nc, in_maps, core_ids=list(range(N_CORES)), trace=_trace
    )
    out_l = np.empty((N, C, H, W), np.float32)
    out_g = np.empty((N, C, H, W), np.float32)
    for core in range(N_CORES):
        r0 = core * ROWS
        out_l[:, :, r0:r0 + ROWS, :] = res.results[core]["out_l"].reshape(
            N, C, ROWS, W)
        out_g[:, :, r0:r0 + ROWS, :] = res.results[core]["out_g"].reshape(
            N, C, ROWS, W)
    if _trace:
        kernel.last_result = res
    return out_l, out_g



# revision 5
# speedup vs baseline: 1.8713x; 1.8713x over previous
"""Trainium2 Bass kernel for nn_CrossAttention (dense_transformer).

Reference computation (N=4, C=512, H=W=32, NH=8 heads):
  k/q/v = 3x3 conv(x) for both branches (l, g); heads split the flattened
  spatial dim into 8 chunks of 128; attention uses channels (512) as the
  sequence axis and the 128 spatial positions as the head dim; then a 1x1
  conv and a scaled residual add.

Sharding: spatial. Head h covers image rows 4h..4h+3, so sharding H into
8 blocks of 4 rows makes every head fully core-local: core h computes the
convs for its 4 output rows (6 input rows with halo), its head's full
attention, the 1x1 conv, and the residual -- zero cross-core communication.

Per-core kernel design (v3):
  * The six 3x3 convs dominate (~86% of matmul columns).  They run in
    fp8e4 with perf_mode=DoubleRow: the contraction dim doubles to 256
    (cin pairs interleaved on the partition axis), halving the matmul
    count vs bf16.  Host pre-quantizes x (scale 16) and conv weights
    (scale 1024) to fp8e4 (clipped to +-240 = TRN max normal).
  * Q/K convs use x as the STATIONARY operand (lhsT = 9 pre-shifted fp8
    copies of x, one per tap, prepared on host) and weights as the moving
    operand, producing the transposed [spatial, channel] layout the
    QK^T matmul needs directly -- no PE transposes at all.
  * V convs use weights stationary / shifted-x moving, producing the
    normal [channel, spatial] layout the AV matmul needs.
  * The QK^T matmul groups are interleaved between the V-conv cout blocks
    so the PE stays dense while the scalar engine works through the
    softmax exps (prevents a HAM clock-down during the attention phase).
  * x and weights stream in per-batch / per-tap-half chunks ordered to
    match first use, so the PE starts ~3us into the kernel.
  * The fp8 descale (1/(sx*sw))^2 and the 1/sqrt(128) attention scale are
    folded into the softmax exp's input scale; V's descale is applied in
    its psum->sbuf copy; resweight is folded into the 1x1 conv weights.
  * Softmax without max-subtraction (logits within +-20, exp safe in
    fp32->bf16) and without an explicit row-sum pass: V is augmented with
    a ones column so the AV matmul's last column is the denominator.
  * QK / AV / 1x1 matmuls stay bf16 (only ~14% of PE columns; keeps the
    accuracy budget for the fp8 convs).
"""

import numpy as np
import ml_dtypes

N, C, H, W, NH = 4, 512, 32, 32, 8
P = 128
ROWS = H // NH          # 4 output rows per core
S = ROWS * W            # 128 spatial positions per core (= head dim)
CK = C // P             # 4 cout chunks
NQ = 2                  # cin 256-pairs (DoubleRow contraction groups)
N_CORES = 8

SX = 16.0               # fp8 scale for x
SW = 1024.0             # fp8 scale for conv weights
INV = 1.0 / (SX * SW)
EXP_SCALE = float(INV * INV / np.sqrt(S))

_BUILT = {}


def _build_bass():
    import concourse.tile as tile
    import concourse.mybir as mybir
    from concourse import bacc

    f32 = mybir.dt.float32
    bf16 = mybir.dt.bfloat16
    f8 = mybir.dt.float8e4
    AF = mybir.ActivationFunctionType
    DR = mybir.MatmulPerfMode.DoubleRow

    nc = bacc.Bacc("TRN2", target_bir_lowering=False)

    # 9 tap-shifted fp8 copies of x: [p, n, tap, q, j, s]; cin = q*256+j*128+p
    xs_l_d = nc.dram_tensor("xs_l", [P, N, 9, NQ, 2, S], f8, kind="ExternalInput")
    xs_g_d = nc.dram_tensor("xs_g", [P, N, 9, NQ, 2, S], f8, kind="ExternalInput")
    # residual x, fp32, [p, n, ck, s]; cout = ck*128+p
    xres_l_d = nc.dram_tensor("xres_l", [P, N, CK, S], f32, kind="ExternalInput")
    xres_g_d = nc.dram_tensor("xres_g", [P, N, CK, S], f32, kind="ExternalInput")
    # conv weights fp8: [conv, p, tap, q, j, cout]
    # conv order: 0=Wk1(k_l) 1=Wq1(q_l) 2=Wk2(k_g) 3=Wq2(q_g) 4=Wv1(v_l) 5=Wv2(v_g)
    w3_d = nc.dram_tensor("w3", [6, P, 9, NQ, 2, C], f8, kind="ExternalInput")
    # wp[0]=Wp1*rw (local branch), wp[1]=Wp2*rw; layout [cin, cout] bf16
    wp_d = nc.dram_tensor("wp", [2, C, C], bf16, kind="ExternalInput")
    outl_d = nc.dram_tensor("out_l", [N, C, S], f32, kind="ExternalOutput")
    outg_d = nc.dram_tensor("out_g", [N, C, S], f32, kind="ExternalOutput")

    with tile.TileContext(nc) as tc:
        with (
            tc.tile_pool(name="singles", bufs=1) as singles,
            tc.tile_pool(name="wpool", bufs=3) as wpool,
            tc.tile_pool(name="attp", bufs=8) as attp,
            tc.tile_pool(name="small", bufs=8) as small,
            tc.tile_pool(name="ps", bufs=5, space="PSUM") as ps_pool,
            tc.tile_pool(name="psy", bufs=3, space="PSUM") as psy_pool,
        ):
            # ---- x loads, one DMA per batch so conv n=0 starts after ~0.6MB
            xs = {}
            for nm, td in (("l", xs_l_d), ("g", xs_g_d)):
                t = singles.tile([P, N, 9, NQ, 2, S], f8, tag=f"xs_{nm}", name=f"xs_{nm}")
                for n in range(N):
                    nc.sync.dma_start(t[:, n], td[:, n])
                xs[nm] = t

            def dma_w3(conv_idx):
                """fp8 weight tile halves (taps 0-4 / 5-8) for one conv."""
                wa = wpool.tile([P, 5, NQ, 2, C], f8, tag="w3a", name="w3a")
                wb = wpool.tile([P, 4, NQ, 2, C], f8, tag="w3b", name="w3b")
                da = lambda: nc.scalar.dma_start(wa, w3_d[conv_idx, :, 0:5])
                db = lambda: nc.scalar.dma_start(wb, w3_d[conv_idx, :, 5:9])
                return (wa, wb), da, db

            def wtap(w, tap):
                wa, wb = w
                return wa[:, tap] if tap < 5 else wb[:, tap - 5]

            def conv_pair(kidx, qidx, xsrc, kdst, qdst):
                """K+Q 3x3 convs sharing the stationary x windows.

                Output layout [s, n, c] (transposed), fp8 DoubleRow."""
                wk, ka, kb = dma_w3(kidx)
                wq, qa, qb = dma_w3(qidx)
                ka(); qa(); kb(); qb()     # interleave halves: K/Q taps 0-4 first
                for n in range(N):
                    pk = ps_pool.tile([P, C], f32, tag="ps", name="ps")
                    pq = ps_pool.tile([P, C], f32, tag="ps", name="ps")
                    for tap in range(9):
                        for q in range(NQ):
                            lhsT = xsrc[:, n, tap, q]              # [P, 2, 128]
                            first = tap == 0 and q == 0
                            last = tap == 8 and q == NQ - 1
                            nc.tensor.matmul(
                                pk, lhsT=lhsT, rhs=wtap(wk, tap)[:, q],
                                start=first, stop=last, perf_mode=DR,
                            )
                            nc.tensor.matmul(
                                pq, lhsT=lhsT, rhs=wtap(wq, tap)[:, q],
                                start=first, stop=last, perf_mode=DR,
                            )
                    nc.vector.tensor_copy(out=kdst[:, n, :], in_=pk)
                    nc.vector.tensor_copy(out=qdst[:, n, :], in_=pq)

            def conv_v_co(wv, xsrc, vdst, co):
                """One cout block of a V 3x3 conv ([c, n, s] layout, DR)."""
                pv = ps_pool.tile([P, C], f32, tag="ps", name="ps")
                for tap in range(9):
                    for q in range(NQ):
                        nc.tensor.matmul(
                            pv,
                            lhsT=wtap(wv, tap)[:, q, :, co * P:(co + 1) * P],
                            rhs=xsrc[:, :, tap, q].rearrange("p n j s -> p j n s"),
                            start=(tap == 0 and q == 0),
                            stop=(tap == 8 and q == NQ - 1),
                            perf_mode=DR,
                        )
                # descale by 1/(sx*sw) while copying out of psum
                nc.vector.tensor_scalar_mul(
                    vdst[:, co, :, 0:S],
                    pv.rearrange("p (n s) -> p n s", n=N),
                    INV,
                )

            def qk_group(q_sb, k_sb, n, pexp):
                """QK^T + scaled exp for one batch; fills pexp [ck, cq]."""
                for ck in range(CK):
                    psum = ps_pool.tile([P, C], f32, tag="ps", name="ps")
                    nc.tensor.matmul(
                        psum,
                        lhsT=k_sb[:, n, ck * P:(ck + 1) * P],
                        rhs=q_sb[:, n, :],
                        start=True,
                        stop=True,
                    )
                    nc.scalar.activation(
                        out=pexp[:, ck, :], in_=psum, func=AF.Exp,
                        scale=EXP_SCALE,
                    )

            def attend_av(pexps, vaug, wp_idx, xres, out_d, out_tag):
                y_sb = singles.tile([P, CK, N, S], bf16, tag=f"y_{out_tag}", name=f"y_{out_tag}")
                wp_sb = singles.tile([P, CK, C], bf16, tag=f"wp_{out_tag}", name=f"wp_{out_tag}")
                nc.sync.dma_start(
                    wp_sb, wp_d[wp_idx].rearrange("(ck p) co -> p ck co", p=P)
                )
                # cq outer: y[:, cq] finishes early so the 1x1 conv's first
                # accumulations overlap the remaining AV matmuls
                for cq in range(CK):
                    for n in range(N):
                        pexp = pexps[n]
                        psy = psy_pool.tile([P, S + 1], f32, tag="psy", name="psy")
                        for ck in range(CK):
                            nc.tensor.matmul(
                                psy,
                                lhsT=pexp[:, ck, cq * P:(cq + 1) * P],
                                rhs=vaug[:, ck, n, :],
                                start=(ck == 0),
                                stop=(ck == CK - 1),
                            )
                        rec = small.tile([P, 1], f32, tag="rec", name="rec")
                        nc.vector.reciprocal(rec, psy[:, S:S + 1])
                        nc.vector.tensor_scalar_mul(
                            y_sb[:, cq, n, :], psy[:, 0:S], rec,
                        )
                # 1x1 conv (weights pre-scaled by resweight) + residual add;
                # per-co output DMA so the store overlaps the remaining 1x1s
                out_sb = singles.tile([P, N, CK, S], f32, tag=f"out_{out_tag}", name=f"out_{out_tag}")
                outv = out_d.rearrange("n (co p) s -> p n co s", p=P)
                for co in range(CK):
                    psum = ps_pool.tile([P, C], f32, tag="ps", name="ps")
                    for ci in range(CK):
                        nc.tensor.matmul(
                            psum,
                            lhsT=wp_sb[:, ci, co * P:(co + 1) * P],
                            rhs=y_sb[:, ci].rearrange("p n s -> p (n s)"),
                            start=(ci == 0),
                            stop=(ci == CK - 1),
                        )
                    nc.vector.tensor_add(
                        out=out_sb[:, :, co],
                        in0=psum.rearrange("p (n s) -> p n s", n=N),
                        in1=xres[:, :, co],
                    )
                    nc.sync.dma_start(outv[:, :, co], out_sb[:, :, co])

            # persistent attention operands
            k1 = singles.tile([P, N, C], bf16, tag="k1", name="k1")   # k_l [s, n, c]
            q1 = singles.tile([P, N, C], bf16, tag="q1", name="q1")   # q_l
            k2 = singles.tile([P, N, C], bf16, tag="k2", name="k2")   # k_g
            q2 = singles.tile([P, N, C], bf16, tag="q2", name="q2")   # q_g
            vl = singles.tile([P, CK, N, S + 1], bf16, tag="vl", name="vl")
            vg = singles.tile([P, CK, N, S + 1], bf16, tag="vg", name="vg")
            nc.vector.memset(vl[:, :, :, S:S + 1], 1.0)
            nc.vector.memset(vg[:, :, :, S:S + 1], 1.0)

            # branch A: global queries attend over local k/v -> out_g
            # branch B: local queries attend over global k/v -> out_l
            conv_pair(0, 1, xs["l"], k1, q1)
            conv_pair(2, 3, xs["g"], k2, q2)

            # residual x arrives mid-kernel, ahead of the projections
            xres = {}
            for nm, td in (("g", xres_g_d), ("l", xres_l_d)):
                t = singles.tile([P, N, CK, S], f32, tag=f"xres_{nm}", name=f"xres_{nm}")
                nc.sync.dma_start(t, td[:, 0:N])
                xres[nm] = t

            # V_l conv with the QK^T groups interleaved between cout blocks:
            # the PE stays dense while the scalar engine runs the exps.
            pexps_a = [attp.tile([P, CK, C], bf16, tag="pexp_a", name="pexp_a", bufs=4)
                       for _ in range(N)]
            pexps_b = [attp.tile([P, CK, C], bf16, tag="pexp_b", name="pexp_b", bufs=4)
                       for _ in range(N)]
            wv_l, va, vb = dma_w3(4)
            va(); vb()
            conv_v_co(wv_l, xs["l"], vl, 0)
            qk_group(q2, k1, 0, pexps_a[0])
            qk_group(q2, k1, 1, pexps_a[1])
            conv_v_co(wv_l, xs["l"], vl, 1)
            qk_group(q2, k1, 2, pexps_a[2])
            qk_group(q2, k1, 3, pexps_a[3])
            conv_v_co(wv_l, xs["l"], vl, 2)
            qk_group(q1, k2, 0, pexps_b[0])
            qk_group(q1, k2, 1, pexps_b[1])
            conv_v_co(wv_l, xs["l"], vl, 3)
            qk_group(q1, k2, 2, pexps_b[2])
            qk_group(q1, k2, 3, pexps_b[3])

            attend_av(pexps_a, vl, 1, xres["g"], outg_d, "g")
            wv_g, va, vb = dma_w3(5)
            va(); vb()
            for co in range(CK):
                conv_v_co(wv_g, xs["g"], vg, co)
            attend_av(pexps_b, vg, 0, xres["l"], outl_d, "l")

    nc.finalize()
    return nc


def _prep_host_inputs(x_l, x_g, Wk1, Wq1, Wv1, Wk2, Wq2, Wv2, Wp1, Wp2, resweight):
    """Build the 8 per-core input maps (numpy) from full inputs."""
    bf = ml_dtypes.bfloat16
    f8 = ml_dtypes.float8_e4m3
    rw = np.float32(np.asarray(resweight))

    def q8(a, s):
        return np.clip(np.asarray(a, np.float32) * s, -240, 240).astype(f8)

    def wconv(w):
        # [cout, cin, 3, 3] -> [p, tap, q, j, cout] with cin = q*256+j*128+p
        wq = q8(w, SW).reshape(C, NQ, 2, P, 3, 3)
        return np.ascontiguousarray(
            wq.transpose(3, 4, 5, 1, 2, 0).reshape(P, 9, NQ, 2, C)
        )

    w3 = np.stack([wconv(w) for w in (Wk1, Wq1, Wk2, Wq2, Wv1, Wv2)])
    wp = np.stack([
        np.ascontiguousarray(np.asarray(Wp1, np.float32)[:, :, 0, 0].T * rw),
        np.ascontiguousarray(np.asarray(Wp2, np.float32)[:, :, 0, 0].T * rw),
    ]).astype(bf)  # [2, C(in), C(out)]

    x_l = np.asarray(x_l, np.float32)
    x_g = np.asarray(x_g, np.float32)
    # quantized + padded, cin split as [q, j, p]
    xq = {
        "l": q8(np.pad(x_l, ((0, 0), (0, 0), (1, 1), (1, 1))), SX)
        .reshape(N, NQ, 2, P, H + 2, W + 2),
        "g": q8(np.pad(x_g, ((0, 0), (0, 0), (1, 1), (1, 1))), SX)
        .reshape(N, NQ, 2, P, H + 2, W + 2),
    }
    # residual layout [p, n, ck, s] with cout = ck*128+p
    xr = {
        "l": x_l.reshape(N, CK, P, H, W),
        "g": x_g.reshape(N, CK, P, H, W),
    }

    in_maps = []
    for core in range(N_CORES):
        r0 = core * ROWS
        m = {"w3": w3, "wp": wp}
        for nm in ("l", "g"):
            xs9 = np.empty((P, N, 9, NQ, 2, S), f8)
            for ky in range(3):
                for kx in range(3):
                    win = xq[nm][:, :, :, :, r0 + ky:r0 + ky + ROWS, kx:kx + W]
                    xs9[:, :, ky * 3 + kx] = win.transpose(3, 0, 1, 2, 4, 5).reshape(
                        P, N, NQ, 2, S)
            m[f"xs_{nm}"] = xs9
            m[f"xres_{nm}"] = np.ascontiguousarray(
                xr[nm][:, :, :, r0:r0 + ROWS, :].transpose(2, 0, 1, 3, 4).reshape(
                    P, N, CK, S))
        in_maps.append(m)
    return in_maps


def kernel(x_l, x_g, Wk1, Wq1, Wv1, Wk2, Wq2, Wv2, Wp1, Wp2, resweight,
           _trace=False):
    from concourse.bass_utils import run_bass_kernel_spmd

    if "nc" not in _BUILT:
        _BUILT["nc"] = _build_bass()
    nc = _BUILT["nc"]

    in_maps = _prep_host_inputs(
        x_l, x_g, Wk1, Wq1, Wv1, Wk2, Wq2, Wv2, Wp1, Wp2, resweight
    )
    res = run_bass_kernel_spmd(
        nc, in_maps, core_ids=list(range(N_CORES)), trace=_trace
    )
    out_l = np.empty((N, C, H, W), np.float32)
    out_g = np.empty((N, C, H, W), np.float32)
    for core in range(N_CORES):
        r0 = core * ROWS
        out_l[:, :, r0:r0 + ROWS, :] = res.results[core]["out_l"].reshape(
            N, C, ROWS, W)
        out_g[:, :, r0:r0 + ROWS, :] = res.results[core]["out_g"].reshape(
            N, C, ROWS, W)
    if _trace:
        kernel.last_result = res
    return out_l, out_g
